# revision 14
# baseline (speedup 1.0000x reference)
"""Trainium2 Bass kernel for nn_HermesMessageLayer (gnn_message_passing).

Math: out[e,i,n] = sum_{b,f,r,j,m} inp[e,j,m] * precomp[e,f,r]
                                   * kernel[b,f,n,m] * weight[b,r,i,j] + bias[i]

Staging (per core, data-parallel over E across 8 cores):
  KW[(j,m), (ni, fr)] = sum_b kernel[b,f,n,m]*weight[b,r,i,j]   (host, tiny;
                        columns ordered ni-major / fr-innermost)
  t[e, ni, fr] = inp[e,(j,m)] @ KW                              (PE matmul)
  out[e, ni]   = sum_fr pc[e,fr] * t[e,ni,fr]                   (one custom
                 DVE mul-cumsum op per tile + a Pool strided diff)
  bias is added on the host during unpack.

Per 128-edge tile on device:
  - inp rows transpose-loaded (XBAR DMA, bf16, rows padded 96->128) so the
    contraction dim (j,m) lands on SBUF partitions for the matmul stationary.
  - one PE matmul pair (N=480 x2, two PSUM banks) computes t for 128 edges.
  - one custom DVE op (MUL_CUMSUM_ANT: scan(ADD, Src0*Src1)) reads t straight
    from PSUM (fp32) and pc via a stride-0 broadcast AP, writing the running
    per-(ni,fr) product cumsum S to SBUF in fp32.
  - Pool (gpsimd) computes the per-page sums acc[ni] = S[10(ni+1)] - S[10 ni]
    with strided APs (leading S column memset to 0), cast to bf16.
  - bf16 HWDGE store to a partition-major layout; host un-permutes + upcasts
    and adds bias.
"""

import sys

import numpy as np

sys.path.insert(0, "/opt/trn_rl_repo")

import ml_dtypes

import concourse.bass as bass
import concourse.bacc as bacc
import concourse.tile as tile
from concourse import mybir
from concourse.bass_utils import run_bass_kernel_spmd

# ---- custom DVE op: out[k] = cumsum_k(in0[k] * in1[k]) --------------------
from concourse import dve_ops
from concourse.dve_spec import Spec, Src0, Src1, scan, AluOp, lower
from concourse.dve_uop import DveOpSpec


def _mul_cumsum_ref(in0, in1, s0, s1, imm2):
    p = in0.shape[0]
    prod = in0.astype(np.float32).reshape(p, -1) * in1.astype(np.float32).reshape(
        p, -1
    )
    return np.cumsum(prod, axis=1)


_MUL_CUMSUM_SPEC = Spec(body=scan(AluOp.ADD, Src0 * Src1), reference=_mul_cumsum_ref)
_OP_NAME = "MUL_CUMSUM_ANT"


def _register_mul_cumsum():
    if _OP_NAME in dve_ops._SUB_OPCODE_FOR_NAME:
        return next(o for o in dve_ops.OPS if o.name == _OP_NAME)
    row = dve_ops._CUSTOM_DVE_ROW_BASE + len(dve_ops.OPS)
    shas = {
        ver: DveOpSpec(
            name=_OP_NAME, opcode=row, uops=lower(_MUL_CUMSUM_SPEC, ver=ver), rd1_en=True
        ).sha(ver)
        for ver in ("v3", "v4")
    }
    op = dve_ops.DveOp(_OP_NAME, _MUL_CUMSUM_SPEC, subdim=False, uops_sha=shas)
    dve_ops.OPS.append(op)
    dve_ops._SUB_OPCODE_FOR_NAME[_OP_NAME] = row
    dve_ops.CUSTOM_DVE_SPECS[_OP_NAME] = _MUL_CUMSUM_SPEC
    return op


MUL_CUMSUM = _register_mul_cumsum()

# Problem dims
E, J, I = 300000, 32, 32
M, N = 3, 3
B, F, R = 6, 5, 2
JM = J * M          # 96
NI = I * N          # 96  (col layout is (i, n): ni = i*3 + n)
FR = F * R          # 10
TCOLS = FR * NI     # 960

NCORES = 8
E_CORE = E // NCORES            # 37500
G = 16                          # tiles per group
TILE_E = 128                    # edges per tile (PSUM partitions)
GROUP_E = G * TILE_E            # 2048
NG = -(-E_CORE // GROUP_E)      # 19 groups
E_PAD = NG * GROUP_E            # 38912

BF16 = mybir.dt.bfloat16
F32 = mybir.dt.float32


N_ACT = 3                       # tiles per group handled by the ScalarE
                                # pipeline (the rest go through the DVE scan)
WARM_MM = 12                    # back-to-back warmup matmuls (~4.8 us cold)
N_SCAN_BUFS = 3                 # manually rotated cumsum buffers (col 0
                                # pre-zeroed once; scan writes cols 1..960)


def build_program(ng: int = NG, n_act: int = N_ACT):
    """Build the single-core Bass program (same program runs SPMD on all cores)."""
    nc = bacc.Bacc("TRN2", target_bir_lowering=False, debug=False)

    # ScalarE-pipeline tiles, spread through the group so PSUM buffers and
    # the Pool queue stay evenly loaded.
    stride = G // n_act if n_act else G + 1
    act_set = {k * stride + stride - 1 for k in range(n_act)}

    e_pad = ng * GROUP_E
    inp_t = nc.dram_tensor("inp_aug", [e_pad, 128], BF16, kind="ExternalInput").ap()
    pc_t = nc.dram_tensor("pc", [ng, 128, G, FR], F32, kind="ExternalInput").ap()
    kw_t = nc.dram_tensor("kw", [JM, TCOLS], BF16, kind="ExternalInput").ap()
    out_t = nc.dram_tensor("out", [ng, 128, G, NI], BF16, kind="ExternalOutput").ap()

    with tile.TileContext(nc) as tc:
        with (
            tc.tile_pool(name="const", bufs=1) as const_pool,
            tc.tile_pool(name="inpT", bufs=2) as inpT_pool,
            tc.tile_pool(name="pc", bufs=2) as pc_pool,
            tc.tile_pool(name="u", bufs=2) as u_pool,
            tc.tile_pool(name="w", bufs=2) as w_pool,
            tc.tile_pool(name="acc", bufs=2) as acc_pool,
            tc.tile_pool(name="psum", bufs=3, space="PSUM") as psum_pool,
            tc.tile_pool(name="warm", bufs=1, space="PSUM") as warm_pool,
        ):
            kw_sb = const_pool.tile([JM, TCOLS], BF16)
            nc.sync.dma_start(kw_sb[:], kw_t[:])

            # Manually rotated scan buffers: col 0 is zeroed once and only
            # ever read (the scan writes cols 1..TCOLS), so the per-tile
            # page-diff's first page reads an exact zero with no per-tile
            # memset on any engine.
            scans = [
                const_pool.tile([128, TCOLS + 1], F32, name=f"scan{i}")
                for i in range(N_SCAN_BUFS)
            ]
            for sb in scans:
                nc.gpsimd.memset(sb[:, 0:1], 0.0)

            # HAM warmup: ~5 us of contiguous PE activity releases the clock
            # gate (K=4/8 -> 8/8) before the steady-state loop begins.
            warm = warm_pool.tile([128, 512], F32)
            for _ in range(WARM_MM):
                nc.tensor.matmul(
                    warm[:, 0:480],
                    kw_sb[:, 0:128],
                    kw_sb[:, 0:480],
                    start=True,
                    stop=True,
                )

            for g in range(ng):
                inpT = inpT_pool.tile([128, GROUP_E], BF16)
                nc.sync.dma_start(
                    inpT[:],
                    inp_t[g * GROUP_E : (g + 1) * GROUP_E, :],
                    transpose=True,
                )
                pc = pc_pool.tile([128, G, FR], F32)
                nc.sync.dma_start(pc[:], pc_t[g])
                acc = acc_pool.tile([128, G, NI], BF16)

                for gi in range(G):
                    ps = psum_pool.tile([128, 1024], F32)
                    lhsT = inpT[0:JM, gi * TILE_E : (gi + 1) * TILE_E]
                    nc.tensor.matmul(
                        ps[:, 0:480], lhsT, kw_sb[:, 0:480], start=True, stop=True
                    )
                    nc.tensor.matmul(
                        ps[:, 512:992], lhsT, kw_sb[:, 480:960], start=True, stop=True
                    )
                    ps_b = ps[:].rearrange("p (b x) -> p b x", b=2)

                    if gi in act_set:
                        # ScalarE pipeline: 10 per-partition-scaled copies
                        # (ACT, PSUM-direct) + a Pool add tree.
                        u = u_pool.tile([128, FR, NI], BF16)
                        for fr in range(FR):
                            nc.scalar.mul(
                                u[:, fr],
                                ps_b[:, :, fr:480:FR],
                                pc[:, gi, fr : fr + 1],
                            )
                        w = w_pool.tile([128, 6, NI], BF16)
                        nc.gpsimd.tensor_add(w[:, 0:5], u[:, 0:5], u[:, 5:10])
                        nc.gpsimd.tensor_add(w[:, 5], w[:, 0], w[:, 1])
                        nc.gpsimd.tensor_add(w[:, 5], w[:, 5], w[:, 2])
                        nc.gpsimd.tensor_add(w[:, 5], w[:, 5], w[:, 3])
                        nc.gpsimd.tensor_add(acc[:, gi], w[:, 5], w[:, 4])
                    else:
                        # DVE pipeline: one fused mul-cumsum over (ni, fr)
                        # PSUM-direct, then a Pool strided page-diff.
                        s = scans[(g * G + gi) % N_SCAN_BUFS]
                        pc_b = (
                            pc[:, gi]
                            .rearrange("p (o fr) -> p o fr", o=1)
                            .broadcast_to([128, NI, FR])
                        )
                        nc.vector._custom_dve(
                            MUL_CUMSUM,
                            out=s[:, 1 : TCOLS + 1],
                            in0=ps_b[:, :, 0:480],
                            in1=pc_b,
                        )
                        # acc[ni] = S[10(ni+1)] - S[10 ni]: per-ni page sums
                        nc.gpsimd.tensor_sub(
                            acc[:, gi],
                            s[:, FR : TCOLS + 1 : FR],
                            s[:, 0:TCOLS:FR],
                        )

                nc.sync.dma_start(out_t[g], acc[:])

    nc.compile()
    return nc


def _pack_core(inp_c, precomp_c, ng: int = NG):
    """Pack one core's slice into the padded/permuted device layouts."""
    e_pad = ng * GROUP_E
    e_c = inp_c.shape[0]
    inp_aug = np.zeros([e_pad, 128], dtype=ml_dtypes.bfloat16)
    inp_aug[:e_c, :JM] = inp_c.reshape(e_c, JM).astype(ml_dtypes.bfloat16)

    pc_pad = np.zeros([e_pad, FR], dtype=np.float32)
    pc_pad[:e_c] = precomp_c.reshape(e_c, FR)
    # tile (g, gi) partition p holds edge g*GROUP_E + gi*TILE_E + p
    pc_perm = np.ascontiguousarray(
        pc_pad.reshape(ng, G, TILE_E, FR).transpose(0, 2, 1, 3)
    )
    return inp_aug, pc_perm


def _pack_shared(kernel, weight):
    # KW[(j,m), (i,n,f,r)] = sum_b kernel[b,f,n,m] * weight[b,r,i,j]
    # column order: ni-major, fr-innermost  (col = ni*FR + fr)
    kw = np.einsum(
        "bfnm,brij->jminfr",
        kernel.astype(np.float64),
        weight.astype(np.float64),
    ).reshape(JM, TCOLS)
    return kw.astype(ml_dtypes.bfloat16)


_PROGRAM_CACHE = {}


def _get_program(ng: int = NG, n_act: int = N_ACT):
    key = (ng, n_act)
    if key not in _PROGRAM_CACHE:
        _PROGRAM_CACHE[key] = build_program(ng, n_act)
    return _PROGRAM_CACHE[key]


def kernel(inp, precomp, kernel, weight, bias):
    inp = np.asarray(inp)
    precomp = np.asarray(precomp)
    kernel_np = np.asarray(kernel)
    weight = np.asarray(weight)
    bias = np.asarray(bias)

    kw_b = _pack_shared(kernel_np, weight)

    in_maps = []
    for c in range(NCORES):
        sl = slice(c * E_CORE, (c + 1) * E_CORE)
        inp_aug, pc_perm = _pack_core(inp[sl], precomp[sl])
        in_maps.append({"inp_aug": inp_aug, "pc": pc_perm, "kw": kw_b})

    nc = _get_program()
    res = run_bass_kernel_spmd(nc, in_maps, list(range(NCORES)))

    out = np.empty([E, I, N], dtype=np.float32)
    for c in range(NCORES):
        o = np.asarray(res.results[c]["out"]).astype(np.float32)  # [NG,128,G,NI]
        o = o.transpose(0, 2, 1, 3).reshape(NG * GROUP_E, NI)[:E_CORE]
        out[c * E_CORE : (c + 1) * E_CORE] = o.reshape(E_CORE, I, N)
    out += bias.astype(np.float32)[None, :, None]
    return out


# revision 38
# speedup vs baseline: 1.4602x; 1.4602x over previous
"""Trainium2 Bass kernel for nn_HermesMessageLayer (gnn_message_passing).

Math: out[e,i,n] = sum_{b,f,r,j,m} inp[e,j,m] * precomp[e,f,r]
                                   * kernel[b,f,n,m] * weight[b,r,i,j] + bias[i]

Staging (per core, data-parallel over E across 8 cores):
  KW[(j,m), (ni, fr)] = sum_b kernel[b,f,n,m]*weight[b,r,i,j]   (host, tiny;
                        columns ordered ni-major / fr-innermost)
  t[e, ni, fr] = inp[e,(j,m)] @ KW                              (PE matmul)
  out[e, ni]   = sum_fr pc[e,fr] * t[e,ni,fr]                   (one custom
                 DVE mul-cumsum op per tile + a Pool strided diff)
  bias is added on the host during unpack.

Per 128-edge tile on device:
  - inp rows transpose-loaded (XBAR DMA, bf16, rows padded 96->128) so the
    contraction dim (j,m) lands on SBUF partitions for the matmul stationary.
  - one PE matmul pair (N=480 x2, two PSUM banks) computes t for 128 edges;
    a ~5us warmup matmul burst releases the HAM clock gate first.
  - 3 of 16 tiles/group (PSUM-direct path): MUL_CUMSUM_ANT (custom DVE
    scan(ADD, Src0*Src1), 1x) reads t from PSUM fp32 with pc via a stride-0
    broadcast AP, writing the running cumsum S (fp32) to a rotating buffer
    whose col 0 is pre-zeroed once; Pool then takes the strided page-diff
    acc[ni] = S[10(ni+1)] - S[10 ni].
  - 13 of 16 tiles/group (2x path): ScalarE copies t PSUM->SBUF bf16; the
    hand-written 2x perf-mode op PAGED_MUL_SCAN_ANT (pair-products +
    pairsum + per-page-reset scan, 2 elem/cycle) leaves exact page sums at
    odd positions (fr=9); Pool extracts them with a strided copy.
  - bf16 HWDGE store to a partition-major layout; host un-permutes + upcasts
    and adds bias.
"""

import sys

import numpy as np

sys.path.insert(0, "/opt/trn_rl_repo")

import ml_dtypes

import concourse.bass as bass
import concourse.bacc as bacc
import concourse.tile as tile
from concourse import mybir
from concourse.bass_utils import run_bass_kernel_spmd

# ---- custom DVE ops -------------------------------------------------------
# MUL_CUMSUM_ANT: out[k] = cumsum_k(in0[k] * in1[k])   (flat, 1x, PSUM-ok)
# PAGED_MUL_SCAN_ANT: per-page-reset mul-scan with a hand-written 2x
#   perf-mode program (pairs: p_lo+p_hi per cycle). Page ends (odd k) hold
#   the exact per-page sums; bf16 SBUF source required for 2x.
from concourse import dve_ops
from concourse.dve_spec import Spec, Src0, Src1, scan, AluOp as SAluOp, lower
from concourse.dve_uop import (
    DveOpSpec,
    UopConfig,
    InpSel,
    OutPath,
    OutSel,
    AluOp,
    AluInp,
    DelayInp,
    Trigger,
)


def _mul_cumsum_ref(in0, in1, s0, s1, imm2):
    p = in0.shape[0]
    prod = in0.astype(np.float32).reshape(p, -1) * in1.astype(np.float32).reshape(
        p, -1
    )
    return np.cumsum(prod, axis=1)


_MUL_CUMSUM_SPEC = Spec(body=scan(SAluOp.ADD, Src0 * Src1), reference=_mul_cumsum_ref)
_OP_NAME = "MUL_CUMSUM_ANT"


def _register_mul_cumsum():
    if _OP_NAME in dve_ops._SUB_OPCODE_FOR_NAME:
        return next(o for o in dve_ops.OPS if o.name == _OP_NAME)
    row = dve_ops._CUSTOM_DVE_ROW_BASE + len(dve_ops.OPS)
    shas = {
        ver: DveOpSpec(
            name=_OP_NAME, opcode=row, uops=lower(_MUL_CUMSUM_SPEC, ver=ver), rd1_en=True
        ).sha(ver)
        for ver in ("v3", "v4")
    }
    op = dve_ops.DveOp(_OP_NAME, _MUL_CUMSUM_SPEC, subdim=False, uops_sha=shas)
    dve_ops.OPS.append(op)
    dve_ops._SUB_OPCODE_FOR_NAME[_OP_NAME] = row
    dve_ops.CUSTOM_DVE_SPECS[_OP_NAME] = _MUL_CUMSUM_SPEC
    return op


MUL_CUMSUM = _register_mul_cumsum()

_PG_NAME = "PAGED_MUL_SCAN_ANT"


def _paged_ref(in0, in1, s0, s1, imm2):
    p = in0.shape[0]
    prod = in0.astype(np.float32).reshape(p, -1, 10) * in1.astype(
        np.float32
    ).reshape(p, -1, 10)
    return np.cumsum(prod, axis=2).reshape(p, -1)


_PAGED_SPEC = Spec(body=scan(SAluOp.ADD, Src0 * Src1), reference=_paged_ref)


def _bypass_tail(u, first, lanes):
    for b in range(first, 8):
        u.datapath_config[b].pass_through_alu()
        u.datapath_config[b].pass_through_delay(*lanes)
    return u


def _paged_fsm(u, kind, write_hi):
    if kind == "seed":
        u.trigger = (Trigger.COUNT, Trigger.NONE, Trigger.NONE)
        u.next_uop = (1, 0, 0)
        u.repeat_count = 1
        return u
    u.require_inp0 = 1
    u.require_inp1 = 1
    u.enable_output(OutSel.ALU_OUT, OutPath.WR0_LO)
    if write_hi:
        u.enable_output(OutSel.ALU_OUT, OutPath.WR0_HI)
    if kind == "steady":
        u.trigger = (Trigger.SRC_TENSOR_DONE, Trigger.SUB_DIM_DONE, Trigger.NONE)
        u.next_uop = (0, 2, 0)
    else:  # step: consumes the first element/pair of a page with reset
        u.trigger = (Trigger.SRC_TENSOR_DONE, Trigger.SUB_DIM_DONE, Trigger.COUNT)
        u.next_uop = (0, 2, 1)
        u.repeat_count = 1
    return u


def _paged_1x():
    uops = []
    for kind in ("seed", "steady", "step"):
        u = UopConfig()
        u.enable_input(InpSel.SRC_0, 1)
        u.enable_input(InpSel.SRC_1, 2)
        u.enable_input(InpSel.ZERO, 3)
        u.datapath_config[0].enable_alu(
            AluOp.MULTIPLY, AluInp.PREV_DELAY_0, AluInp.PREV_DELAY_1
        ).pass_through_delay(0, 1, 2)
        d1 = u.datapath_config[1]
        if kind == "seed":
            d1.enable_alu(AluOp.BYPASS, AluInp.PREV_DELAY_2, AluInp.PREV_DELAY_2)
        elif kind == "steady":
            d1.enable_alu(AluOp.ADD, AluInp.CURR_ALU_OUT, AluInp.PREV_ALU_OUT)
        else:
            d1.enable_alu(AluOp.BYPASS, AluInp.PREV_ALU_OUT, AluInp.PREV_ALU_OUT)
        d1.pass_through_delay(0, 1, 2)
        _bypass_tail(u, 2, (0, 1, 2))
        uops.append(_paged_fsm(u, kind, write_hi=False))
    return uops


def _paged_2x():
    uops = []
    for kind in ("seed", "steady", "step"):
        u = UopConfig()
        u.enable_input(InpSel.SRC_0, 1)
        u.enable_input(InpSel.SRC_1, 2)
        u.enable_input(InpSel.SRC_0_HI, 3)
        u.enable_input(InpSel.SRC_1_HI, 4)
        u.enable_input(InpSel.ZERO, 5)
        u.datapath_config[0].enable_alu(
            AluOp.MULTIPLY, AluInp.PREV_DELAY_0, AluInp.PREV_DELAY_1
        ).pass_through_delay(0, 1, 2, 3, 4)
        d1 = u.datapath_config[1]
        d1.enable_alu(AluOp.MULTIPLY, AluInp.PREV_DELAY_2, AluInp.PREV_DELAY_3)
        d1.enable_delay_from_src(DelayInp.PREV_ALU_OUT, 0)
        d1.pass_through_delay(1, 2, 3, 4)
        u.datapath_config[2].enable_alu(
            AluOp.ADD, AluInp.PREV_ALU_OUT, AluInp.PREV_DELAY_0
        ).pass_through_delay(0, 1, 2, 3, 4)
        d3 = u.datapath_config[3]
        if kind == "seed":
            d3.enable_alu(AluOp.BYPASS, AluInp.PREV_DELAY_4, AluInp.PREV_DELAY_4)
        elif kind == "steady":
            d3.enable_alu(AluOp.ADD, AluInp.CURR_ALU_OUT, AluInp.PREV_ALU_OUT)
        else:
            d3.enable_alu(AluOp.BYPASS, AluInp.PREV_ALU_OUT, AluInp.PREV_ALU_OUT)
        d3.pass_through_delay(0, 1, 2, 3, 4)
        _bypass_tail(u, 4, (0, 1, 2, 3, 4))
        uops.append(_paged_fsm(u, kind, write_hi=True))
    return uops


def _register_paged():
    if _PG_NAME in dve_ops._SUB_OPCODE_FOR_NAME:
        return next(o for o in dve_ops.OPS if o.name == _PG_NAME)
    row = dve_ops._CUSTOM_DVE_ROW_BASE + len(dve_ops.OPS)
    op = dve_ops.DveOp(
        _PG_NAME, _PAGED_SPEC, subdim=True, uops_sha={"v3": "x", "v4": "x"}
    )
    dve_ops.OPS.append(op)
    dve_ops._SUB_OPCODE_FOR_NAME[_PG_NAME] = row
    dve_ops.CUSTOM_DVE_SPECS[_PG_NAME] = _PAGED_SPEC
    for ver in ("v3", "v4"):
        hand = DveOpSpec(
            name=_PG_NAME,
            opcode=row,
            uops=_paged_1x(),
            uops_2x=_paged_2x(),
            perf_max=1,
            rd1_en=True,
        )
        hand.validate(ver)
        dve_ops._COMPILE_CACHE[(_PG_NAME, ver)] = hand
    return op


PAGED_MUL_SCAN = _register_paged()

# Problem dims
E, J, I = 300000, 32, 32
M, N = 3, 3
B, F, R = 6, 5, 2
JM = J * M          # 96
NI = I * N          # 96  (col layout is (i, n): ni = i*3 + n)
FR = F * R          # 10
TCOLS = FR * NI     # 960

NCORES = 8
E_CORE = E // NCORES            # 37500
G = 16                          # tiles per group
TILE_E = 128                    # edges per tile (PSUM partitions)
GROUP_E = G * TILE_E            # 2048
NG = -(-E_CORE // GROUP_E)      # 19 groups
E_PAD = NG * GROUP_E            # 38912
N_TILES = -(-E_CORE // TILE_E)  # 293 tiles actually needed
G_LAST = N_TILES - (NG - 1) * G  # 5 tiles in the final (short) group

BF16 = mybir.dt.bfloat16
F32 = mybir.dt.float32


N_PSUM = 3                      # tiles per group on the PSUM-direct 1x scan
                                # path; the rest use ACT copy + 2x paged scan
WARM_MM = 12                    # back-to-back warmup matmuls (~4.8 us cold)
N_SCAN_BUFS = 3                 # manually rotated cumsum buffers (col 0
                                # pre-zeroed once; scan writes cols 1..960)


def build_program(ng: int = NG, n_psum: int = N_PSUM):
    """Build the single-core Bass program (same program runs SPMD on all cores)."""
    nc = bacc.Bacc("TRN2", target_bir_lowering=False, debug=False)

    # PSUM-direct tiles, spread through the group so PSUM buffers and the
    # Pool queue stay evenly loaded.
    stride = G // n_psum if n_psum else G + 1
    psum_set = {k * stride for k in range(n_psum)}

    e_pad = ng * GROUP_E
    inp_t = nc.dram_tensor("inp_aug", [e_pad, 128], BF16, kind="ExternalInput").ap()
    pc_t = nc.dram_tensor("pc", [ng, 128, G, FR], BF16, kind="ExternalInput").ap()
    kw_t = nc.dram_tensor("kw", [JM, TCOLS], BF16, kind="ExternalInput").ap()
    out_t = nc.dram_tensor("out", [ng, 128, G, NI], BF16, kind="ExternalOutput").ap()

    with tile.TileContext(nc) as tc:
        with (
            tc.tile_pool(name="const", bufs=1) as const_pool,
            tc.tile_pool(name="inpT", bufs=2) as inpT_pool,
            tc.tile_pool(name="pc", bufs=2) as pc_pool,
            tc.tile_pool(name="tsb", bufs=3) as tsb_pool,
            tc.tile_pool(name="w", bufs=3) as w_pool,
            tc.tile_pool(name="acc", bufs=3) as acc_pool,
            tc.tile_pool(name="psum", bufs=3, space="PSUM") as psum_pool,
            tc.tile_pool(name="warm", bufs=1, space="PSUM") as warm_pool,
        ):
            kw_sb = const_pool.tile([JM, TCOLS], BF16)
            nc.sync.dma_start(kw_sb[:], kw_t[:])

            # Manually rotated scan buffers: col 0 is zeroed once and only
            # ever read (the scan writes cols 1..TCOLS), so the per-tile
            # page-diff's first page reads an exact zero with no per-tile
            # memset on any engine.
            scans = [
                const_pool.tile([128, TCOLS + 1], F32, name=f"scan{i}")
                for i in range(N_SCAN_BUFS)
            ]
            for sb in scans:
                nc.gpsimd.memset(sb[:, 0:1], 0.0)

            # HAM warmup: ~5 us of contiguous PE activity releases the clock
            # gate (K=4/8 -> 8/8) before the steady-state loop begins.
            warm = warm_pool.tile([128, 512], F32)
            for _ in range(WARM_MM):
                nc.tensor.matmul(
                    warm[:, 0:480],
                    kw_sb[:, 0:128],
                    kw_sb[:, 0:480],
                    start=True,
                    stop=True,
                )

            for g in range(ng):
                tg = G if g < ng - 1 else G_LAST
                inpT = inpT_pool.tile([128, GROUP_E], BF16)
                nc.sync.dma_start(
                    inpT[:, 0 : tg * TILE_E],
                    inp_t[g * GROUP_E : g * GROUP_E + tg * TILE_E, :],
                    transpose=True,
                )
                pc = pc_pool.tile([128, G, FR], BF16)
                nc.sync.dma_start(pc[:], pc_t[g])
                acc = acc_pool.tile([128, G, NI], BF16)

                for gi in range(tg):
                    ps = psum_pool.tile([128, 1024], F32)
                    lhsT = inpT[0:JM, gi * TILE_E : (gi + 1) * TILE_E]
                    nc.tensor.matmul(
                        ps[:, 0:480], lhsT, kw_sb[:, 0:480], start=True, stop=True
                    )
                    nc.tensor.matmul(
                        ps[:, 512:992], lhsT, kw_sb[:, 480:960], start=True, stop=True
                    )
                    ps_b = ps[:].rearrange("p (b x) -> p b x", b=2)
                    pc_b = (
                        pc[:, gi]
                        .rearrange("p (o fr) -> p o fr", o=1)
                        .broadcast_to([128, NI, FR])
                    )

                    if gi in psum_set:
                        # PSUM-direct: one fused mul-cumsum over (ni, fr),
                        # then a Pool strided page-diff.
                        s = scans[(g * G + gi) % N_SCAN_BUFS]
                        nc.vector._custom_dve(
                            MUL_CUMSUM,
                            out=s[:, 1 : TCOLS + 1],
                            in0=ps_b[:, :, 0:480],
                            in1=pc_b,
                        )
                        # acc[ni] = S[10(ni+1)] - S[10 ni]: per-ni page sums
                        nc.gpsimd.tensor_sub(
                            acc[:, gi],
                            s[:, FR : TCOLS + 1 : FR],
                            s[:, 0:TCOLS:FR],
                        )
                    else:
                        # ACT copies t to SBUF bf16; the hand-built 2x paged
                        # scan leaves per-page sums at odd positions (fr=9);
                        # Pool extracts them strided.
                        tsb = tsb_pool.tile([128, TCOLS], BF16)
                        tsb_view = tsb[:].rearrange("p (b x) -> p b x", b=2)
                        nc.scalar.copy(tsb_view, ps_b[:, :, 0:480])
                        w = w_pool.tile([128, TCOLS], BF16)
                        inst = nc.vector._custom_dve(
                            PAGED_MUL_SCAN,
                            out=w[:],
                            in0=tsb[:].rearrange("p (ni fr) -> p ni fr", fr=FR),
                            in1=pc_b,
                        )
                        inst.ins.perf_max = 1
                        nc.gpsimd.tensor_copy(
                            acc[:, gi],
                            w[:].rearrange("p (ni fr) -> p ni fr", fr=FR)[
                                :, :, FR - 1
                            ],
                        )

                nc.sync.dma_start(out_t[g][:, 0:tg], acc[:, 0:tg])

    nc.compile()
    return nc


def _pack_core(inp_c, precomp_c, ng: int = NG):
    """Pack one core's slice into the padded/permuted device layouts."""
    e_pad = ng * GROUP_E
    e_c = inp_c.shape[0]
    inp_aug = np.zeros([e_pad, 128], dtype=ml_dtypes.bfloat16)
    inp_aug[:e_c, :JM] = inp_c.reshape(e_c, JM).astype(ml_dtypes.bfloat16)

    pc_pad = np.zeros([e_pad, FR], dtype=np.float32)
    pc_pad[:e_c] = precomp_c.reshape(e_c, FR)
    # tile (g, gi) partition p holds edge g*GROUP_E + gi*TILE_E + p
    pc_perm = np.ascontiguousarray(
        pc_pad.reshape(ng, G, TILE_E, FR).transpose(0, 2, 1, 3)
    ).astype(ml_dtypes.bfloat16)
    return inp_aug, pc_perm


def _pack_shared(kernel, weight):
    # KW[(j,m), (i,n,f,r)] = sum_b kernel[b,f,n,m] * weight[b,r,i,j]
    # column order: ni-major, fr-innermost  (col = ni*FR + fr)
    kw = np.einsum(
        "bfnm,brij->jminfr",
        kernel.astype(np.float64),
        weight.astype(np.float64),
    ).reshape(JM, TCOLS)
    return kw.astype(ml_dtypes.bfloat16)


_PROGRAM_CACHE = {}


def _get_program(ng: int = NG, n_psum: int = N_PSUM):
    key = (ng, n_psum)
    if key not in _PROGRAM_CACHE:
        _PROGRAM_CACHE[key] = build_program(ng, n_psum)
    return _PROGRAM_CACHE[key]


def kernel(inp, precomp, kernel, weight, bias):
    inp = np.asarray(inp)
    precomp = np.asarray(precomp)
    kernel_np = np.asarray(kernel)
    weight = np.asarray(weight)
    bias = np.asarray(bias)

    kw_b = _pack_shared(kernel_np, weight)

    in_maps = []
    for c in range(NCORES):
        sl = slice(c * E_CORE, (c + 1) * E_CORE)
        inp_aug, pc_perm = _pack_core(inp[sl], precomp[sl])
        in_maps.append({"inp_aug": inp_aug, "pc": pc_perm, "kw": kw_b})

    nc = _get_program()
    res = run_bass_kernel_spmd(nc, in_maps, list(range(NCORES)))

    out = np.empty([E, I, N], dtype=np.float32)
    for c in range(NCORES):
        o = np.asarray(res.results[c]["out"]).astype(np.float32)  # [NG,128,G,NI]
        o = o.transpose(0, 2, 1, 3).reshape(NG * GROUP_E, NI)[:E_CORE]
        out[c * E_CORE : (c + 1) * E_CORE] = o.reshape(E_CORE, I, N)
    out += bias.astype(np.float32)[None, :, None]
    return out


# revision 42
# speedup vs baseline: 1.5062x; 1.0315x over previous
"""Trainium2 Bass kernel for nn_HermesMessageLayer (gnn_message_passing).

Math: out[e,i,n] = sum_{b,f,r,j,m} inp[e,j,m] * precomp[e,f,r]
                                   * kernel[b,f,n,m] * weight[b,r,i,j] + bias[i]

Staging (per core, data-parallel over E across 8 cores):
  KW[(j,m), (ni, fr)] = sum_b kernel[b,f,n,m]*weight[b,r,i,j]   (host, tiny;
                        columns ordered ni-major / fr-innermost)
  t[e, ni, fr] = inp[e,(j,m)] @ KW                              (PE matmul)
  out[e, ni]   = sum_fr pc[e,fr] * t[e,ni,fr]                   (one custom
                 DVE mul-cumsum op per tile + a Pool strided diff)
  bias is added on the host during unpack.

Per 128-edge tile on device:
  - inp rows transpose-loaded (XBAR DMA, bf16, rows padded 96->128) so the
    contraction dim (j,m) lands on SBUF partitions for the matmul stationary.
  - one PE matmul pair (N=480 x2, two PSUM banks) computes t for 128 edges;
    a ~5us warmup matmul burst releases the HAM clock gate first.
  - 3 of 16 tiles/group (PSUM-direct path): MUL_CUMSUM_ANT (custom DVE
    scan(ADD, Src0*Src1), 1x) reads t from PSUM fp32 with pc via a stride-0
    broadcast AP, writing the running cumsum S (fp32) to a rotating buffer
    whose col 0 is pre-zeroed once; Pool then takes the strided page-diff
    acc[ni] = S[10(ni+1)] - S[10 ni].
  - 13 of 16 tiles/group (2x path): ScalarE copies t PSUM->SBUF bf16; the
    hand-written 2x perf-mode op PAGED_MUL_SCAN_ANT (pair-products +
    pairsum + per-page-reset scan, 2 elem/cycle) leaves exact page sums at
    odd positions (fr=9); Pool extracts them with a strided copy.
  - bf16 HWDGE store to a partition-major layout; host un-permutes + upcasts
    and adds bias.
"""

import sys

import numpy as np

sys.path.insert(0, "/opt/trn_rl_repo")

import ml_dtypes

import concourse.bass as bass
import concourse.bacc as bacc
import concourse.tile as tile
from concourse import mybir
from concourse.bass_utils import run_bass_kernel_spmd

# ---- custom DVE ops -------------------------------------------------------
# MUL_CUMSUM_ANT: out[k] = cumsum_k(in0[k] * in1[k])   (flat, 1x, PSUM-ok)
# PAGED_MUL_SCAN_ANT: per-page-reset mul-scan with a hand-written 2x
#   perf-mode program (pairs: p_lo+p_hi per cycle). Page ends (odd k) hold
#   the exact per-page sums; bf16 SBUF source required for 2x.
from concourse import dve_ops
from concourse.dve_spec import Spec, Src0, Src1, scan, AluOp as SAluOp, lower
from concourse.dve_uop import (
    DveOpSpec,
    UopConfig,
    InpSel,
    OutPath,
    OutSel,
    AluOp,
    AluInp,
    DelayInp,
    Trigger,
)


def _mul_cumsum_ref(in0, in1, s0, s1, imm2):
    p = in0.shape[0]
    prod = in0.astype(np.float32).reshape(p, -1) * in1.astype(np.float32).reshape(
        p, -1
    )
    return np.cumsum(prod, axis=1)


_MUL_CUMSUM_SPEC = Spec(body=scan(SAluOp.ADD, Src0 * Src1), reference=_mul_cumsum_ref)
_OP_NAME = "MUL_CUMSUM_ANT"


def _register_mul_cumsum():
    if _OP_NAME in dve_ops._SUB_OPCODE_FOR_NAME:
        return next(o for o in dve_ops.OPS if o.name == _OP_NAME)
    row = dve_ops._CUSTOM_DVE_ROW_BASE + len(dve_ops.OPS)
    shas = {
        ver: DveOpSpec(
            name=_OP_NAME, opcode=row, uops=lower(_MUL_CUMSUM_SPEC, ver=ver), rd1_en=True
        ).sha(ver)
        for ver in ("v3", "v4")
    }
    op = dve_ops.DveOp(_OP_NAME, _MUL_CUMSUM_SPEC, subdim=False, uops_sha=shas)
    dve_ops.OPS.append(op)
    dve_ops._SUB_OPCODE_FOR_NAME[_OP_NAME] = row
    dve_ops.CUSTOM_DVE_SPECS[_OP_NAME] = _MUL_CUMSUM_SPEC
    return op


MUL_CUMSUM = _register_mul_cumsum()

_PG_NAME = "PAGED_MUL_SCAN_ANT"


def _paged_ref(in0, in1, s0, s1, imm2):
    p = in0.shape[0]
    prod = in0.astype(np.float32).reshape(p, -1, 10) * in1.astype(
        np.float32
    ).reshape(p, -1, 10)
    return np.cumsum(prod, axis=2).reshape(p, -1)


_PAGED_SPEC = Spec(body=scan(SAluOp.ADD, Src0 * Src1), reference=_paged_ref)


def _bypass_tail(u, first, lanes):
    for b in range(first, 8):
        u.datapath_config[b].pass_through_alu()
        u.datapath_config[b].pass_through_delay(*lanes)
    return u


def _paged_fsm(u, kind, write_hi):
    if kind == "seed":
        u.trigger = (Trigger.COUNT, Trigger.NONE, Trigger.NONE)
        u.next_uop = (1, 0, 0)
        u.repeat_count = 1
        return u
    u.require_inp0 = 1
    u.require_inp1 = 1
    u.enable_output(OutSel.ALU_OUT, OutPath.WR0_LO)
    if write_hi:
        u.enable_output(OutSel.ALU_OUT, OutPath.WR0_HI)
    if kind == "steady":
        u.trigger = (Trigger.SRC_TENSOR_DONE, Trigger.SUB_DIM_DONE, Trigger.NONE)
        u.next_uop = (0, 2, 0)
    else:  # step: consumes the first element/pair of a page with reset
        u.trigger = (Trigger.SRC_TENSOR_DONE, Trigger.SUB_DIM_DONE, Trigger.COUNT)
        u.next_uop = (0, 2, 1)
        u.repeat_count = 1
    return u


def _paged_1x():
    uops = []
    for kind in ("seed", "steady", "step"):
        u = UopConfig()
        u.enable_input(InpSel.SRC_0, 1)
        u.enable_input(InpSel.SRC_1, 2)
        u.enable_input(InpSel.ZERO, 3)
        u.datapath_config[0].enable_alu(
            AluOp.MULTIPLY, AluInp.PREV_DELAY_0, AluInp.PREV_DELAY_1
        ).pass_through_delay(0, 1, 2)
        d1 = u.datapath_config[1]
        if kind == "seed":
            d1.enable_alu(AluOp.BYPASS, AluInp.PREV_DELAY_2, AluInp.PREV_DELAY_2)
        elif kind == "steady":
            d1.enable_alu(AluOp.ADD, AluInp.CURR_ALU_OUT, AluInp.PREV_ALU_OUT)
        else:
            d1.enable_alu(AluOp.BYPASS, AluInp.PREV_ALU_OUT, AluInp.PREV_ALU_OUT)
        d1.pass_through_delay(0, 1, 2)
        _bypass_tail(u, 2, (0, 1, 2))
        uops.append(_paged_fsm(u, kind, write_hi=False))
    return uops


def _paged_2x():
    uops = []
    for kind in ("seed", "steady", "step"):
        u = UopConfig()
        u.enable_input(InpSel.SRC_0, 1)
        u.enable_input(InpSel.SRC_1, 2)
        u.enable_input(InpSel.SRC_0_HI, 3)
        u.enable_input(InpSel.SRC_1_HI, 4)
        u.enable_input(InpSel.ZERO, 5)
        u.datapath_config[0].enable_alu(
            AluOp.MULTIPLY, AluInp.PREV_DELAY_0, AluInp.PREV_DELAY_1
        ).pass_through_delay(0, 1, 2, 3, 4)
        d1 = u.datapath_config[1]
        d1.enable_alu(AluOp.MULTIPLY, AluInp.PREV_DELAY_2, AluInp.PREV_DELAY_3)
        d1.enable_delay_from_src(DelayInp.PREV_ALU_OUT, 0)
        d1.pass_through_delay(1, 2, 3, 4)
        u.datapath_config[2].enable_alu(
            AluOp.ADD, AluInp.PREV_ALU_OUT, AluInp.PREV_DELAY_0
        ).pass_through_delay(0, 1, 2, 3, 4)
        d3 = u.datapath_config[3]
        if kind == "seed":
            d3.enable_alu(AluOp.BYPASS, AluInp.PREV_DELAY_4, AluInp.PREV_DELAY_4)
        elif kind == "steady":
            d3.enable_alu(AluOp.ADD, AluInp.CURR_ALU_OUT, AluInp.PREV_ALU_OUT)
        else:
            d3.enable_alu(AluOp.BYPASS, AluInp.PREV_ALU_OUT, AluInp.PREV_ALU_OUT)
        d3.pass_through_delay(0, 1, 2, 3, 4)
        _bypass_tail(u, 4, (0, 1, 2, 3, 4))
        uops.append(_paged_fsm(u, kind, write_hi=True))
    return uops


def _register_paged():
    if _PG_NAME in dve_ops._SUB_OPCODE_FOR_NAME:
        return next(o for o in dve_ops.OPS if o.name == _PG_NAME)
    row = dve_ops._CUSTOM_DVE_ROW_BASE + len(dve_ops.OPS)
    op = dve_ops.DveOp(
        _PG_NAME, _PAGED_SPEC, subdim=True, uops_sha={"v3": "x", "v4": "x"}
    )
    dve_ops.OPS.append(op)
    dve_ops._SUB_OPCODE_FOR_NAME[_PG_NAME] = row
    dve_ops.CUSTOM_DVE_SPECS[_PG_NAME] = _PAGED_SPEC
    for ver in ("v3", "v4"):
        hand = DveOpSpec(
            name=_PG_NAME,
            opcode=row,
            uops=_paged_1x(),
            uops_2x=_paged_2x(),
            perf_max=1,
            rd1_en=True,
        )
        hand.validate(ver)
        dve_ops._COMPILE_CACHE[(_PG_NAME, ver)] = hand
    return op


PAGED_MUL_SCAN = _register_paged()

# Problem dims
E, J, I = 300000, 32, 32
M, N = 3, 3
B, F, R = 6, 5, 2
JM = J * M          # 96
NI = I * N          # 96  (col layout is (i, n): ni = i*3 + n)
FR = F * R          # 10
TCOLS = FR * NI     # 960

NCORES = 8
E_CORE = E // NCORES            # 37500
G = 16                          # tiles per group
TILE_E = 128                    # edges per tile (PSUM partitions)
GROUP_E = G * TILE_E            # 2048
NG = -(-E_CORE // GROUP_E)      # 19 groups
E_PAD = NG * GROUP_E            # 38912
N_TILES = -(-E_CORE // TILE_E)  # 293 tiles actually needed
G_LAST = N_TILES - (NG - 1) * G  # 5 tiles in the final (short) group

BF16 = mybir.dt.bfloat16
F32 = mybir.dt.float32


N_PSUM = 3                      # tiles per group on the PSUM-direct 1x scan
                                # path; the rest use ACT copy + 2x paged scan
WARM_MM = 12                    # back-to-back warmup matmuls (~4.8 us cold)
N_SCAN_BUFS = 3                 # manually rotated cumsum buffers (col 0
                                # pre-zeroed once; scan writes cols 1..960)


def build_program(ng: int = NG, n_psum: int = N_PSUM):
    """Build the single-core Bass program (same program runs SPMD on all cores)."""
    nc = bacc.Bacc("TRN2", target_bir_lowering=False, debug=False)

    # PSUM-direct tiles, spread through the group so PSUM buffers and the
    # Pool queue stay evenly loaded.
    stride = G // n_psum if n_psum else G + 1
    psum_set = {k * stride for k in range(n_psum)}

    inp_t = nc.dram_tensor(
        "inp_aug", [ng, 128, GROUP_E], BF16, kind="ExternalInput"
    ).ap()
    pc_t = nc.dram_tensor("pc", [ng, 128, G, FR], BF16, kind="ExternalInput").ap()
    kw_t = nc.dram_tensor("kw", [JM, TCOLS], BF16, kind="ExternalInput").ap()
    out_t = nc.dram_tensor("out", [ng, 128, G, NI], BF16, kind="ExternalOutput").ap()

    with tile.TileContext(nc) as tc:
        with (
            tc.tile_pool(name="const", bufs=1) as const_pool,
            tc.tile_pool(name="inpT", bufs=2) as inpT_pool,
            tc.tile_pool(name="pc", bufs=2) as pc_pool,
            tc.tile_pool(name="tsb", bufs=4) as tsb_pool,
            tc.tile_pool(name="w", bufs=4) as w_pool,
            tc.tile_pool(name="acc", bufs=3) as acc_pool,
            tc.tile_pool(name="psum", bufs=3, space="PSUM") as psum_pool,
            tc.tile_pool(name="warm", bufs=1, space="PSUM") as warm_pool,
        ):
            kw_sb = const_pool.tile([JM, TCOLS], BF16)
            nc.sync.dma_start(kw_sb[:], kw_t[:])

            # Manually rotated scan buffers: col 0 is zeroed once and only
            # ever read (the scan writes cols 1..TCOLS), so the per-tile
            # page-diff's first page reads an exact zero with no per-tile
            # memset on any engine.
            scans = [
                const_pool.tile([128, TCOLS + 1], F32, name=f"scan{i}")
                for i in range(N_SCAN_BUFS)
            ]
            for sb in scans:
                nc.gpsimd.memset(sb[:, 0:1], 0.0)

            # HAM warmup: ~5 us of contiguous PE activity releases the clock
            # gate (K=4/8 -> 8/8) before the steady-state loop begins.
            warm = warm_pool.tile([128, 512], F32)
            for _ in range(WARM_MM):
                nc.tensor.matmul(
                    warm[:, 0:480],
                    kw_sb[:, 0:128],
                    kw_sb[:, 0:480],
                    start=True,
                    stop=True,
                )

            for g in range(ng):
                tg = G if g < ng - 1 else G_LAST
                inpT = inpT_pool.tile([128, GROUP_E], BF16)
                nc.sync.dma_start(
                    inpT[:, 0 : tg * TILE_E],
                    inp_t[g][:, 0 : tg * TILE_E],
                )
                pc = pc_pool.tile([128, G, FR], BF16)
                nc.sync.dma_start(pc[:], pc_t[g])
                acc = acc_pool.tile([128, G, NI], BF16)

                for gi in range(tg):
                    ps = psum_pool.tile([128, 1024], F32)
                    lhsT = inpT[0:JM, gi * TILE_E : (gi + 1) * TILE_E]
                    nc.tensor.matmul(
                        ps[:, 0:480], lhsT, kw_sb[:, 0:480], start=True, stop=True
                    )
                    nc.tensor.matmul(
                        ps[:, 512:992], lhsT, kw_sb[:, 480:960], start=True, stop=True
                    )
                    ps_b = ps[:].rearrange("p (b x) -> p b x", b=2)
                    pc_b = (
                        pc[:, gi]
                        .rearrange("p (o fr) -> p o fr", o=1)
                        .broadcast_to([128, NI, FR])
                    )

                    if gi in psum_set:
                        # PSUM-direct: one fused mul-cumsum over (ni, fr),
                        # then a Pool strided page-diff.
                        s = scans[(g * G + gi) % N_SCAN_BUFS]
                        nc.vector._custom_dve(
                            MUL_CUMSUM,
                            out=s[:, 1 : TCOLS + 1],
                            in0=ps_b[:, :, 0:480],
                            in1=pc_b,
                        )
                        # acc[ni] = S[10(ni+1)] - S[10 ni]: per-ni page sums
                        nc.gpsimd.tensor_sub(
                            acc[:, gi],
                            s[:, FR : TCOLS + 1 : FR],
                            s[:, 0:TCOLS:FR],
                        )
                    else:
                        # ACT copies t to SBUF bf16; the hand-built 2x paged
                        # scan leaves per-page sums at odd positions (fr=9);
                        # Pool extracts them strided.
                        tsb = tsb_pool.tile([128, TCOLS], BF16)
                        tsb_view = tsb[:].rearrange("p (b x) -> p b x", b=2)
                        nc.scalar.copy(tsb_view, ps_b[:, :, 0:480])
                        w = w_pool.tile([128, TCOLS], BF16)
                        inst = nc.vector._custom_dve(
                            PAGED_MUL_SCAN,
                            out=w[:],
                            in0=tsb[:].rearrange("p (ni fr) -> p ni fr", fr=FR),
                            in1=pc_b,
                        )
                        inst.ins.perf_max = 1
                        nc.gpsimd.tensor_copy(
                            acc[:, gi],
                            w[:].rearrange("p (ni fr) -> p ni fr", fr=FR)[
                                :, :, FR - 1
                            ],
                        )

                nc.sync.dma_start(out_t[g][:, 0:tg], acc[:, 0:tg])

    nc.compile()
    return nc


def _pack_core(inp_c, precomp_c, ng: int = NG):
    """Pack one core's slice into the padded/permuted device layouts."""
    e_pad = ng * GROUP_E
    e_c = inp_c.shape[0]
    inp_pad = np.zeros([e_pad, 128], dtype=ml_dtypes.bfloat16)
    inp_pad[:e_c, :JM] = inp_c.reshape(e_c, JM).astype(ml_dtypes.bfloat16)
    # pre-transposed per group: [ng, 128 (jm+pad), GROUP_E] so the device
    # load is a straight HWDGE DMA instead of an XBAR transpose
    inp_aug = np.ascontiguousarray(
        inp_pad.reshape(ng, GROUP_E, 128).transpose(0, 2, 1)
    )

    pc_pad = np.zeros([e_pad, FR], dtype=np.float32)
    pc_pad[:e_c] = precomp_c.reshape(e_c, FR)
    # tile (g, gi) partition p holds edge g*GROUP_E + gi*TILE_E + p
    pc_perm = np.ascontiguousarray(
        pc_pad.reshape(ng, G, TILE_E, FR).transpose(0, 2, 1, 3)
    ).astype(ml_dtypes.bfloat16)
    return inp_aug, pc_perm


def _pack_shared(kernel, weight):
    # KW[(j,m), (i,n,f,r)] = sum_b kernel[b,f,n,m] * weight[b,r,i,j]
    # column order: ni-major, fr-innermost  (col = ni*FR + fr)
    kw = np.einsum(
        "bfnm,brij->jminfr",
        kernel.astype(np.float64),
        weight.astype(np.float64),
    ).reshape(JM, TCOLS)
    return kw.astype(ml_dtypes.bfloat16)


_PROGRAM_CACHE = {}


def _get_program(ng: int = NG, n_psum: int = N_PSUM):
    key = (ng, n_psum)
    if key not in _PROGRAM_CACHE:
        _PROGRAM_CACHE[key] = build_program(ng, n_psum)
    return _PROGRAM_CACHE[key]


def kernel(inp, precomp, kernel, weight, bias):
    inp = np.asarray(inp)
    precomp = np.asarray(precomp)
    kernel_np = np.asarray(kernel)
    weight = np.asarray(weight)
    bias = np.asarray(bias)

    kw_b = _pack_shared(kernel_np, weight)

    in_maps = []
    for c in range(NCORES):
        sl = slice(c * E_CORE, (c + 1) * E_CORE)
        inp_aug, pc_perm = _pack_core(inp[sl], precomp[sl])
        in_maps.append({"inp_aug": inp_aug, "pc": pc_perm, "kw": kw_b})

    nc = _get_program()
    res = run_bass_kernel_spmd(nc, in_maps, list(range(NCORES)))

    out = np.empty([E, I, N], dtype=np.float32)
    for c in range(NCORES):
        o = np.asarray(res.results[c]["out"]).astype(np.float32)  # [NG,128,G,NI]
        o = o.transpose(0, 2, 1, 3).reshape(NG * GROUP_E, NI)[:E_CORE]
        out[c * E_CORE : (c + 1) * E_CORE] = o.reshape(E_CORE, I, N)
    out += bias.astype(np.float32)[None, :, None]
    return out


# revision 46
# speedup vs baseline: 1.5069x; 1.0005x over previous
"""Trainium2 Bass kernel for nn_HermesMessageLayer (gnn_message_passing).

Math: out[e,i,n] = sum_{b,f,r,j,m} inp[e,j,m] * precomp[e,f,r]
                                   * kernel[b,f,n,m] * weight[b,r,i,j] + bias[i]

Staging (per core, data-parallel over E across 8 cores):
  KW[(j,m), (ni, fr)] = sum_b kernel[b,f,n,m]*weight[b,r,i,j]   (host, tiny;
                        columns ordered ni-major / fr-innermost)
  t[e, ni, fr] = inp[e,(j,m)] @ KW                              (PE matmul)
  out[e, ni]   = sum_fr pc[e,fr] * t[e,ni,fr]                   (one custom
                 DVE mul-cumsum op per tile + a Pool strided diff)
  bias is added on the host during unpack.

Per 128-edge tile on device:
  - inp rows transpose-loaded (XBAR DMA, bf16, rows padded 96->128) so the
    contraction dim (j,m) lands on SBUF partitions for the matmul stationary.
  - one PE matmul pair (N=480 x2, two PSUM banks) computes t for 128 edges;
    a ~5us warmup matmul burst releases the HAM clock gate first.
  - 3 of 16 tiles/group (PSUM-direct path): MUL_CUMSUM_ANT (custom DVE
    scan(ADD, Src0*Src1), 1x) reads t from PSUM fp32 with pc via a stride-0
    broadcast AP, writing the running cumsum S (fp32) to a rotating buffer
    whose col 0 is pre-zeroed once; Pool then takes the strided page-diff
    acc[ni] = S[10(ni+1)] - S[10 ni].
  - 13 of 16 tiles/group (2x path): ScalarE copies t PSUM->SBUF bf16; the
    hand-written 2x perf-mode op PAGED_MUL_SCAN_ANT (pair-products +
    pairsum + per-page-reset scan, 2 elem/cycle) leaves exact page sums at
    odd positions (fr=9); Pool extracts them with a strided copy.
  - bf16 HWDGE store to a partition-major layout; host un-permutes + upcasts
    and adds bias.
"""

import sys

import numpy as np

sys.path.insert(0, "/opt/trn_rl_repo")

import ml_dtypes

import concourse.bass as bass
import concourse.bacc as bacc
import concourse.tile as tile
from concourse import mybir
from concourse.bass_utils import run_bass_kernel_spmd

# ---- custom DVE ops -------------------------------------------------------
# MUL_CUMSUM_ANT: out[k] = cumsum_k(in0[k] * in1[k])   (flat, 1x, PSUM-ok)
# PAGED_MUL_SCAN_ANT: per-page-reset mul-scan with a hand-written 2x
#   perf-mode program (pairs: p_lo+p_hi per cycle). Page ends (odd k) hold
#   the exact per-page sums; bf16 SBUF source required for 2x.
from concourse import dve_ops
from concourse.dve_spec import Spec, Src0, Src1, scan, AluOp as SAluOp, lower
from concourse.dve_uop import (
    DveOpSpec,
    UopConfig,
    InpSel,
    OutPath,
    OutSel,
    AluOp,
    AluInp,
    DelayInp,
    Trigger,
)


def _mul_cumsum_ref(in0, in1, s0, s1, imm2):
    p = in0.shape[0]
    prod = in0.astype(np.float32).reshape(p, -1) * in1.astype(np.float32).reshape(
        p, -1
    )
    return np.cumsum(prod, axis=1)


_MUL_CUMSUM_SPEC = Spec(body=scan(SAluOp.ADD, Src0 * Src1), reference=_mul_cumsum_ref)
_OP_NAME = "MUL_CUMSUM_ANT"


def _register_mul_cumsum():
    if _OP_NAME in dve_ops._SUB_OPCODE_FOR_NAME:
        return next(o for o in dve_ops.OPS if o.name == _OP_NAME)
    row = dve_ops._CUSTOM_DVE_ROW_BASE + len(dve_ops.OPS)
    shas = {
        ver: DveOpSpec(
            name=_OP_NAME, opcode=row, uops=lower(_MUL_CUMSUM_SPEC, ver=ver), rd1_en=True
        ).sha(ver)
        for ver in ("v3", "v4")
    }
    op = dve_ops.DveOp(_OP_NAME, _MUL_CUMSUM_SPEC, subdim=False, uops_sha=shas)
    dve_ops.OPS.append(op)
    dve_ops._SUB_OPCODE_FOR_NAME[_OP_NAME] = row
    dve_ops.CUSTOM_DVE_SPECS[_OP_NAME] = _MUL_CUMSUM_SPEC
    return op


MUL_CUMSUM = _register_mul_cumsum()

_PG_NAME = "PAGED_MUL_SCAN_ANT"


def _paged_ref(in0, in1, s0, s1, imm2):
    p = in0.shape[0]
    prod = in0.astype(np.float32).reshape(p, -1, 10) * in1.astype(
        np.float32
    ).reshape(p, -1, 10)
    return np.cumsum(prod, axis=2).reshape(p, -1)


_PAGED_SPEC = Spec(body=scan(SAluOp.ADD, Src0 * Src1), reference=_paged_ref)


def _bypass_tail(u, first, lanes):
    for b in range(first, 8):
        u.datapath_config[b].pass_through_alu()
        u.datapath_config[b].pass_through_delay(*lanes)
    return u


def _paged_fsm(u, kind, write_hi):
    if kind == "seed":
        u.trigger = (Trigger.COUNT, Trigger.NONE, Trigger.NONE)
        u.next_uop = (1, 0, 0)
        u.repeat_count = 1
        return u
    u.require_inp0 = 1
    u.require_inp1 = 1
    u.enable_output(OutSel.ALU_OUT, OutPath.WR0_LO)
    if write_hi:
        u.enable_output(OutSel.ALU_OUT, OutPath.WR0_HI)
    if kind == "steady":
        u.trigger = (Trigger.SRC_TENSOR_DONE, Trigger.SUB_DIM_DONE, Trigger.NONE)
        u.next_uop = (0, 2, 0)
    else:  # step: consumes the first element/pair of a page with reset
        u.trigger = (Trigger.SRC_TENSOR_DONE, Trigger.SUB_DIM_DONE, Trigger.COUNT)
        u.next_uop = (0, 2, 1)
        u.repeat_count = 1
    return u


def _paged_1x():
    uops = []
    for kind in ("seed", "steady", "step"):
        u = UopConfig()
        u.enable_input(InpSel.SRC_0, 1)
        u.enable_input(InpSel.SRC_1, 2)
        u.enable_input(InpSel.ZERO, 3)
        u.datapath_config[0].enable_alu(
            AluOp.MULTIPLY, AluInp.PREV_DELAY_0, AluInp.PREV_DELAY_1
        ).pass_through_delay(0, 1, 2)
        d1 = u.datapath_config[1]
        if kind == "seed":
            d1.enable_alu(AluOp.BYPASS, AluInp.PREV_DELAY_2, AluInp.PREV_DELAY_2)
        elif kind == "steady":
            d1.enable_alu(AluOp.ADD, AluInp.CURR_ALU_OUT, AluInp.PREV_ALU_OUT)
        else:
            d1.enable_alu(AluOp.BYPASS, AluInp.PREV_ALU_OUT, AluInp.PREV_ALU_OUT)
        d1.pass_through_delay(0, 1, 2)
        _bypass_tail(u, 2, (0, 1, 2))
        uops.append(_paged_fsm(u, kind, write_hi=False))
    return uops


def _paged_2x():
    uops = []
    for kind in ("seed", "steady", "step"):
        u = UopConfig()
        u.enable_input(InpSel.SRC_0, 1)
        u.enable_input(InpSel.SRC_1, 2)
        u.enable_input(InpSel.SRC_0_HI, 3)
        u.enable_input(InpSel.SRC_1_HI, 4)
        u.enable_input(InpSel.ZERO, 5)
        u.datapath_config[0].enable_alu(
            AluOp.MULTIPLY, AluInp.PREV_DELAY_0, AluInp.PREV_DELAY_1
        ).pass_through_delay(0, 1, 2, 3, 4)
        d1 = u.datapath_config[1]
        d1.enable_alu(AluOp.MULTIPLY, AluInp.PREV_DELAY_2, AluInp.PREV_DELAY_3)
        d1.enable_delay_from_src(DelayInp.PREV_ALU_OUT, 0)
        d1.pass_through_delay(1, 2, 3, 4)
        u.datapath_config[2].enable_alu(
            AluOp.ADD, AluInp.PREV_ALU_OUT, AluInp.PREV_DELAY_0
        ).pass_through_delay(0, 1, 2, 3, 4)
        d3 = u.datapath_config[3]
        if kind == "seed":
            d3.enable_alu(AluOp.BYPASS, AluInp.PREV_DELAY_4, AluInp.PREV_DELAY_4)
        elif kind == "steady":
            d3.enable_alu(AluOp.ADD, AluInp.CURR_ALU_OUT, AluInp.PREV_ALU_OUT)
        else:
            d3.enable_alu(AluOp.BYPASS, AluInp.PREV_ALU_OUT, AluInp.PREV_ALU_OUT)
        d3.pass_through_delay(0, 1, 2, 3, 4)
        _bypass_tail(u, 4, (0, 1, 2, 3, 4))
        uops.append(_paged_fsm(u, kind, write_hi=True))
    return uops


def _register_paged():
    if _PG_NAME in dve_ops._SUB_OPCODE_FOR_NAME:
        return next(o for o in dve_ops.OPS if o.name == _PG_NAME)
    row = dve_ops._CUSTOM_DVE_ROW_BASE + len(dve_ops.OPS)
    op = dve_ops.DveOp(
        _PG_NAME, _PAGED_SPEC, subdim=True, uops_sha={"v3": "x", "v4": "x"}
    )
    dve_ops.OPS.append(op)
    dve_ops._SUB_OPCODE_FOR_NAME[_PG_NAME] = row
    dve_ops.CUSTOM_DVE_SPECS[_PG_NAME] = _PAGED_SPEC
    for ver in ("v3", "v4"):
        hand = DveOpSpec(
            name=_PG_NAME,
            opcode=row,
            uops=_paged_1x(),
            uops_2x=_paged_2x(),
            perf_max=1,
            rd1_en=True,
        )
        hand.validate(ver)
        dve_ops._COMPILE_CACHE[(_PG_NAME, ver)] = hand
    return op


PAGED_MUL_SCAN = _register_paged()

# Problem dims
E, J, I = 300000, 32, 32
M, N = 3, 3
B, F, R = 6, 5, 2
JM = J * M          # 96
NI = I * N          # 96  (col layout is (i, n): ni = i*3 + n)
FR = F * R          # 10
TCOLS = FR * NI     # 960

NCORES = 8
E_CORE = E // NCORES            # 37500
G = 16                          # tiles per group
TILE_E = 128                    # edges per tile (PSUM partitions)
GROUP_E = G * TILE_E            # 2048
NG = -(-E_CORE // GROUP_E)      # 19 groups
E_PAD = NG * GROUP_E            # 38912
N_TILES = -(-E_CORE // TILE_E)  # 293 tiles actually needed
G_LAST = N_TILES - (NG - 1) * G  # 5 tiles in the final (short) group

BF16 = mybir.dt.bfloat16
F32 = mybir.dt.float32


N_PSUM = 3                      # tiles per group on the PSUM-direct 1x scan
                                # path; the rest use ACT copy + 2x paged scan
WARM_MM = 12                    # back-to-back warmup matmuls (~4.8 us cold)
N_SCAN_BUFS = 3                 # manually rotated cumsum buffers (col 0
                                # pre-zeroed once; scan writes cols 1..960)


def build_program(ng: int = NG, n_psum: int = N_PSUM):
    """Build the single-core Bass program (same program runs SPMD on all cores)."""
    nc = bacc.Bacc("TRN2", target_bir_lowering=False, debug=False)

    # PSUM-direct tiles, spread through the group so PSUM buffers and the
    # Pool queue stay evenly loaded.
    stride = G // n_psum if n_psum else G + 1
    psum_set = {k * stride for k in range(n_psum)}

    inp_t = nc.dram_tensor(
        "inp_aug", [ng, 128, GROUP_E], BF16, kind="ExternalInput"
    ).ap()
    pc_t = nc.dram_tensor("pc", [ng, 128, G, FR], BF16, kind="ExternalInput").ap()
    kw_t = nc.dram_tensor("kw", [JM, TCOLS], BF16, kind="ExternalInput").ap()
    out_t = nc.dram_tensor("out", [ng, 128, G, NI], BF16, kind="ExternalOutput").ap()

    with tile.TileContext(nc) as tc:
        with (
            tc.tile_pool(name="const", bufs=1) as const_pool,
            tc.tile_pool(name="inpT", bufs=2) as inpT_pool,
            tc.tile_pool(name="pc", bufs=2) as pc_pool,
            tc.tile_pool(name="tsb", bufs=4) as tsb_pool,
            tc.tile_pool(name="w", bufs=4) as w_pool,
            tc.tile_pool(name="acc", bufs=3) as acc_pool,
            tc.tile_pool(name="psum", bufs=3, space="PSUM") as psum_pool,
            tc.tile_pool(name="warm", bufs=1, space="PSUM") as warm_pool,
        ):
            kw_sb = const_pool.tile([JM, TCOLS], BF16)
            nc.sync.dma_start(kw_sb[:], kw_t[:])

            # Manually rotated scan buffers: col 0 is zeroed once and only
            # ever read (the scan writes cols 1..TCOLS), so the per-tile
            # page-diff's first page reads an exact zero with no per-tile
            # memset on any engine.
            scans = [
                const_pool.tile([128, TCOLS + 1], F32, name=f"scan{i}")
                for i in range(N_SCAN_BUFS)
            ]
            for sb in scans:
                nc.gpsimd.memset(sb[:, 0:1], 0.0)

            # HAM warmup: ~5 us of contiguous PE activity releases the clock
            # gate (K=4/8 -> 8/8) before the steady-state loop begins.
            warm = warm_pool.tile([128, 512], F32)
            for _ in range(WARM_MM):
                nc.tensor.matmul(
                    warm[:, 0:480],
                    kw_sb[:, 0:128],
                    kw_sb[:, 0:480],
                    start=True,
                    stop=True,
                )

            for g in range(ng):
                tg = G if g < ng - 1 else G_LAST
                inpT = inpT_pool.tile([128, GROUP_E], BF16)
                nc.sync.dma_start(
                    inpT[:, 0 : tg * TILE_E],
                    inp_t[g][:, 0 : tg * TILE_E],
                )
                pc = pc_pool.tile([128, G, FR], BF16)
                nc.sync.dma_start(pc[:], pc_t[g])
                acc = acc_pool.tile([128, G, NI], BF16)

                for gi in range(tg):
                    ps = psum_pool.tile([128, 1024], F32)
                    lhsT = inpT[0:JM, gi * TILE_E : (gi + 1) * TILE_E]
                    nc.tensor.matmul(
                        ps[:, 0:480], lhsT, kw_sb[:, 0:480], start=True, stop=True
                    )
                    nc.tensor.matmul(
                        ps[:, 512:992], lhsT, kw_sb[:, 480:960], start=True, stop=True
                    )
                    ps_b = ps[:].rearrange("p (b x) -> p b x", b=2)
                    pc_b = (
                        pc[:, gi]
                        .rearrange("p (o fr) -> p o fr", o=1)
                        .broadcast_to([128, NI, FR])
                    )

                    if gi in psum_set:
                        # PSUM-direct: one fused mul-cumsum over (ni, fr),
                        # then a Pool strided page-diff.
                        s = scans[(g * G + gi) % N_SCAN_BUFS]
                        nc.vector._custom_dve(
                            MUL_CUMSUM,
                            out=s[:, 1 : TCOLS + 1],
                            in0=ps_b[:, :, 0:480],
                            in1=pc_b,
                        )
                        # acc[ni] = S[10(ni+1)] - S[10 ni]: per-ni page sums
                        nc.gpsimd.tensor_sub(
                            acc[:, gi],
                            s[:, FR : TCOLS + 1 : FR],
                            s[:, 0:TCOLS:FR],
                        )
                    else:
                        # ACT copies t to SBUF bf16; the hand-built 2x paged
                        # scan leaves per-page sums at odd positions (fr=9);
                        # Pool extracts them strided.
                        tsb = tsb_pool.tile([128, TCOLS], BF16)
                        tsb_view = tsb[:].rearrange("p (b x) -> p b x", b=2)
                        nc.scalar.copy(tsb_view, ps_b[:, :, 0:480])
                        w = w_pool.tile([128, TCOLS], BF16)
                        inst = nc.vector._custom_dve(
                            PAGED_MUL_SCAN,
                            out=w[:],
                            in0=tsb[:].rearrange("p (ni fr) -> p ni fr", fr=FR),
                            in1=pc_b,
                        )
                        inst.ins.perf_max = 1
                        nc.gpsimd.tensor_copy(
                            acc[:, gi],
                            w[:].rearrange("p (ni fr) -> p ni fr", fr=FR)[
                                :, :, FR - 1
                            ],
                        )

                nc.sync.dma_start(out_t[g][:, 0:tg], acc[:, 0:tg])

    nc.compile()
    return nc


def _pack_core(inp_c, precomp_c, ng: int = NG):
    """Pack one core's slice into the padded/permuted device layouts."""
    e_pad = ng * GROUP_E
    e_c = inp_c.shape[0]
    inp_pad = np.zeros([e_pad, 128], dtype=ml_dtypes.bfloat16)
    inp_pad[:e_c, :JM] = inp_c.reshape(e_c, JM).astype(ml_dtypes.bfloat16)
    # pre-transposed per group: [ng, 128 (jm+pad), GROUP_E] so the device
    # load is a straight HWDGE DMA instead of an XBAR transpose
    inp_aug = np.ascontiguousarray(
        inp_pad.reshape(ng, GROUP_E, 128).transpose(0, 2, 1)
    )

    pc_pad = np.zeros([e_pad, FR], dtype=np.float32)
    pc_pad[:e_c] = precomp_c.reshape(e_c, FR)
    # tile (g, gi) partition p holds edge g*GROUP_E + gi*TILE_E + p
    pc_perm = np.ascontiguousarray(
        pc_pad.reshape(ng, G, TILE_E, FR).transpose(0, 2, 1, 3)
    ).astype(ml_dtypes.bfloat16)
    return inp_aug, pc_perm


def _pack_shared(kernel, weight):
    # KW[(j,m), (i,n,f,r)] = sum_b kernel[b,f,n,m] * weight[b,r,i,j]
    # column order: ni-major, fr-innermost  (col = ni*FR + fr)
    kw = np.einsum(
        "bfnm,brij->jminfr",
        kernel.astype(np.float64),
        weight.astype(np.float64),
    ).reshape(JM, TCOLS)
    return kw.astype(ml_dtypes.bfloat16)


_PROGRAM_CACHE = {}


def _get_program(ng: int = NG, n_psum: int = N_PSUM):
    key = (ng, n_psum)
    if key not in _PROGRAM_CACHE:
        _PROGRAM_CACHE[key] = build_program(ng, n_psum)
    return _PROGRAM_CACHE[key]


def kernel(inp, precomp, kernel, weight, bias):
    inp = np.asarray(inp)
    precomp = np.asarray(precomp)
    kernel_np = np.asarray(kernel)
    weight = np.asarray(weight)
    bias = np.asarray(bias)

    kw_b = _pack_shared(kernel_np, weight)

    in_maps = []
    for c in range(NCORES):
        sl = slice(c * E_CORE, (c + 1) * E_CORE)
        inp_aug, pc_perm = _pack_core(inp[sl], precomp[sl])
        in_maps.append({"inp_aug": inp_aug, "pc": pc_perm, "kw": kw_b})

    nc = _get_program()
    res = run_bass_kernel_spmd(nc, in_maps, list(range(NCORES)))

    out = np.empty([E, I, N], dtype=np.float32)
    for c in range(NCORES):
        o = np.asarray(res.results[c]["out"]).astype(np.float32)  # [NG,128,G,NI]
        o = o.transpose(0, 2, 1, 3).reshape(NG * GROUP_E, NI)[:E_CORE]
        out[c * E_CORE : (c + 1) * E_CORE] = o.reshape(E_CORE, I, N)
    out += bias.astype(np.float32)[None, :, None]
    return out
